# revision 30
# baseline (speedup 1.0000x reference)
"""Trainium2 Bass kernel for nn_CrossAttention (B=4, C=256, H=W=64).

Sharding: 8 cores = (batch b, query-half h). Host permutes each core's
channel-flattened inputs so the core's own query half occupies columns
0..IH-1 (softmax/attended sums are j-order invariant), letting the
query slice and combine input be SBUF views of the full feature load.

Algebraic restructure: att = sum_j v[:,j] E[j,i] with v = Wv x + bv
factors as Wv (X E) + bv r.  After softmax normalization the bv term is
constant (already folded into bce on host), and Wv commutes out of the
attention sum, so the combine weight becomes Wca_eff = Wc[:,C:] @ Wv
(folded on host) and the attended matmuls contract the raw features:
xe[cin,i] = sum_j x[cin,j] E[j,i].  No Wv projection pass at all; the
host ships x transposed (xt tiles, j on partitions) for the attended
stationary operand.

Per core, per branch:
  q = Wq x_i + bq        [32, 2048] 4x row-replicated bf16
  k = Wk x_f             [32, 4096] 4x row-replicated bf16 (bk dropped:
                         constant-in-j shift is softmax-invariant)
  S^T[j, i] = k_j . q_i  FOUR K=32 score matmuls run concurrently in PE
                         row strips (tile_position 0/32/64/96) into one
                         4-bank PSUM tile
  E = exp(S^T) bf16      one ACT op per 4-chunk slot ([128, 2048])
  r[i] = sum_j E[j, i]   ones-matmuls col-packed 4-per-slot at M=1;
                         fold+broadcast via one mask matmul;
                         1/r = exp(-ln r) on ACT (everything runs out
                         of the pinned natural_log_exp table set)
  xe[c, i] = sum_j xt[j, c] E[j, i];  xen = xe / r
  comb = Wcx x_i + Wca_eff xen + bce ; out[i] = sum_c |comb|
                         (bias-add + sign-mask abs on DVE; ones-matmul)
Pipelining: one flat 64-slot stream (8 blocks x 8 groups); slot s runs
scores/exp of s and attended/r of s-1, so block boundaries cost the
same as interior slots.  Each block's softmax-normalize + combine chain
is deferred into the next block's slots (fold at +1, normalize at +2,
combine halves at +4/+5, output reduce at +6) and runs on ACT/DVE in
the shadow of PE work.  attp banks are freed early by raw bf16 copies;
the 1/r chain lives on ACT so the DVE FIFO never blocks next-block
attended matmuls.  PSUM: 4 score + 2 attended + 1 r + 1 rotating aux.
"""

import numpy as np
import ml_dtypes

import concourse.bass as bass
import concourse.bacc as bacc
import concourse.tile as tile
import concourse.mybir as mybir
from concourse.bass_utils import run_bass_kernel_spmd

B, C, HH, WW = 4, 256, 64, 64
N = HH * WW          # 4096
CQK = 32
IH = N // 2          # 2048 query rows per core
NCORES = 8
NJC = N // 128       # 32 key-dim 128-chunks
NDG = NJC // 4       # 8 slots of 4 key-chunks per block
NIB = IH // 512      # 4 query blocks per branch

F32 = mybir.dt.float32
F32R = mybir.dt.float32r
BF16 = mybir.dt.bfloat16
AF = mybir.ActivationFunctionType
ALU = mybir.AluOpType


def build_program(nc, tc):
    # ---- DRAM I/O ------------------------------------------------------
    dram = {}
    for name, shape, dt in [
        ("x1f", [2, 128, N], BF16), ("x2f", [2, 128, N], BF16),
        ("x1t", [2, 128, 16 * C], BF16), ("x2t", [2, 128, 16 * C], BF16),
        ("wqt", [2, 128, 128], BF16), ("wkt", [2, 128, 128], BF16),
        ("wctx", [2, 128, C], BF16), ("wcae", [2, 128, C], BF16),
        ("bq", [128, 1], F32), ("bce", [128, 2], F32),
        ("mask4", [128, 128], BF16),
    ]:
        dram[name] = nc.dram_tensor(name, shape, dt, kind="ExternalInput").ap()
    out_d = nc.dram_tensor("out", [2, IH], F32, kind="ExternalOutput").ap()

    import contextlib
    with contextlib.ExitStack() as ctx:
        persist = ctx.enter_context(tc.tile_pool(name="persist", bufs=1))

        wq_sb = persist.tile([128, 2, 128], BF16, tag="wq")
        wk_sb = persist.tile([128, 2, 128], BF16, tag="wk")
        wcx_sb = persist.tile([128, 2, C], BF16, tag="wcx")
        wca_sb = persist.tile([128, 2, C], BF16, tag="wca")
        bq_sb = persist.tile([128, 1], F32, tag="bq")
        bce_sb = persist.tile([128, 2], F32, tag="bce")
        ones_bf = persist.tile([128, 1], BF16, tag="ones")
        mask4_sb = persist.tile([128, 128], BF16, tag="mask4")

        x1f_sb = persist.tile([128, 2, N], BF16, tag="x1f")
        x2f_sb = persist.tile([128, 2, N], BF16, tag="x2f")
        xt_sb = [[persist.tile([128, 16, C], BF16, tag=f"xt{i}{h}",
                               name=f"xt{i}{h}") for h in range(2)]
                 for i in range(2)]
        q4_sb = [persist.tile([128, IH], BF16, tag=f"q{i}", name=f"q{i}")
                 for i in range(2)]
        k4_sb = [[persist.tile([128, N // 2], BF16, tag=f"k{i}{h}",
                               name=f"k{i}{h}") for h in range(2)]
                 for i in range(2)]

        # DMA order = need order: weights, x1f (q1/k1), x2t (branch-0
        # attended), x2f (q2/k2), x1t (branch-1 attended, needed late)
        for w, t in [("wqt", wq_sb), ("wkt", wk_sb)]:
            for kc in range(2):
                nc.sync.dma_start(out=t[:, kc, :], in_=dram[w][kc])
        nc.sync.dma_start(out=bq_sb, in_=dram["bq"])
        nc.vector.memset(ones_bf, 1.0)
        # xf transfers sliced at projection granularity so the first
        # matmuls start ~15us earlier; xt rides the gpsimd queue
        # ONE queue, strict priority order: queues do not fair-share the
        # DMA engines, so everything rides sync in first-need order.
        def xf_half(dst, src_name, jh):
            for jb in range(4):
                lo = jh * IH + jb * 512
                for kc in range(2):
                    nc.sync.dma_start(
                        out=dst[:, kc, lo:lo + 512],
                        in_=dram[src_name][kc][:, lo:lo + 512])

        xf_half(x1f_sb, "x1f", 0)
        xf_half(x1f_sb, "x1f", 1)
        for w, t in [("wctx", wcx_sb), ("wcae", wca_sb)]:
            for kc in range(2):
                nc.sync.dma_start(out=t[:, kc, :], in_=dram[w][kc])
        nc.sync.dma_start(out=bce_sb, in_=dram["bce"])
        nc.sync.dma_start(out=mask4_sb, in_=dram["mask4"])
        nc.sync.dma_start(out=xt_sb[1][0].rearrange("p a c -> p (a c)"),
                          in_=dram["x2t"][0])
        xf_half(x2f_sb, "x2f", 0)
        nc.sync.dma_start(out=xt_sb[1][1].rearrange("p a c -> p (a c)"),
                          in_=dram["x2t"][1])
        xf_half(x2f_sb, "x2f", 1)
        for jh in range(2):
            nc.sync.dma_start(out=xt_sb[0][jh].rearrange("p a c -> p (a c)"),
                              in_=dram["x1t"][jh])

        x1i_kc = [x1f_sb[:, kc, 0:IH] for kc in range(2)]

        # ---- phase 1: q/k projections (x1 then x2) --------------------
        with tc.tile_pool(name="ps_kq", bufs=1, space="PSUM") as ps_kq:

            def kq_proj(xf_kc, wt, dst, bias, only_jb=None):
                for jb in range(4):
                    if only_jb is not None and jb != only_jb:
                        continue
                    sl = bass.ts(jb, 512)
                    kp = ps_kq.tile([128, 512], F32, tag="kq", bufs=3,
                                    name="kp")
                    for kc in range(2):
                        nc.tensor.matmul(kp, wt[:, kc, :], xf_kc[kc][:, sl],
                                         start=(kc == 0), stop=(kc == 1))
                    if bias is not None:
                        nc.scalar.activation(dst[:, sl], kp, AF.Identity,
                                             bias=bias)
                    elif jb % 2 == 0:
                        nc.scalar.activation(dst[:, sl], kp, AF.Copy)
                    else:
                        nc.vector.tensor_copy(dst[:, sl], kp)

            half1 = [[x1f_sb[:, kc, jh * IH:(jh + 1) * IH]
                      for kc in range(2)] for jh in range(2)]
            kq_proj(half1[0], wq_sb, q4_sb[0], bq_sb, only_jb=0)
            kq_proj(half1[0], wk_sb, k4_sb[0][0], None, only_jb=0)

        with tc.tile_pool(name="attn_sb", bufs=1) as asb, \
             tc.tile_pool(name="ps_st", bufs=1, space="PSUM") as ps_st, \
             tc.tile_pool(name="ps_att", bufs=1, space="PSUM") as ps_att, \
             tc.tile_pool(name="ps_r", bufs=1, space="PSUM") as ps_r, \
             tc.tile_pool(name="ps_aux", bufs=1, space="PSUM") as ps_aux:

            # ---- phase 2: one flat 64-slot pipeline -------------------
            blocks = [(br, ib) for br in range(2) for ib in range(NIB)]

            class Blk:
                pass

            def scores(bs, dg):
                # two half-tiles + two exp ops: the next slot's first two
                # score strips only wait on the first exp half, so the
                # Scalar engine streams exps back-to-back.
                est2 = []
                for h in range(2):
                    stp = ps_st.tile([128, 2, 512], F32, tag=f"stp{h}",
                                     bufs=1, name=f"stp{h}")
                    for v in range(2):
                        u = 2 * h + v
                        jc = dg * 4 + u
                        jh, jloc = jc // 16, jc % 16
                        nc.tensor.matmul(
                            stp[:, v, :],
                            bs.k4[jh][32 * u:32 * (u + 1),
                                      bass.ts(jloc, 128)],
                            bs.q4[32 * u:32 * (u + 1), bs.isl],
                            start=True, stop=True,
                            tile_position=(32 * u, 0))
                    est = asb.tile([128, 2, 512], BF16, tag=f"est{h}",
                                   bufs=3, name=f"est{h}")
                    nc.scalar.activation(
                        est.rearrange("p a n -> p (a n)"),
                        stp.rearrange("p a n -> p (a n)"), AF.Exp)
                    est2.append(est)
                return est2

            def attended(bs, dg, est2):
                for u in range(4):
                    jc = dg * 4 + u
                    jh, jloc = jc // 16, jc % 16
                    for c2 in range(2):
                        nc.tensor.matmul(
                            bs.attp[c2],
                            bs.xt[jh][:, jloc, bass.ds(c2 * 128, 128)],
                            est2[u // 2][:, u % 2, :],
                            start=(dg == 0 and u == 0),
                            stop=(dg == NDG - 1 and u == 3))
                for u in range(4):
                    nc.tensor.matmul(
                        bs.rp[32 * u:32 * u + 1, :], ones_bf,
                        est2[u // 2][:, u % 2, :],
                        start=(dg == 0), stop=(dg == NDG - 1),
                        tile_position=(0, 32 * u))

            def start_block(br, ib):
                bs = Blk()
                bs.br, bs.ib = br, ib
                bs.isl = bass.ts(ib, 512)
                bs.q4, bs.k4 = q4_sb[br], k4_sb[br]
                bs.xt = xt_sb[1 - br]
                bs.attp = [ps_att.tile([128, 512], F32, tag="attp", bufs=2,
                                       name=f"attp{c2}") for c2 in range(2)]
                bs.rp = ps_r.tile([128, 512], F32, tag="rp", bufs=1,
                                  name="rp")
                return bs

            def end_block(bs, last=False):
                """Issued right after attended(dg7): free rp/attp via DVE."""
                bs.r_sb = asb.tile([128, 512], BF16, tag="rsb", bufs=2,
                                   name="r_sb")
                nc.vector.tensor_copy(bs.r_sb, bs.rp)
                if last:      # nothing follows: normalize attp in place
                    bs.att_raw = bs.attp
                    return
                bs.att_raw = [asb.tile([128, 512], BF16, tag="attraw",
                                       bufs=4, name=f"attraw{c2}")
                              for c2 in range(2)]
                for c2 in range(2):
                    nc.vector.tensor_copy(bs.att_raw[c2], bs.attp[c2])

            def t_fold_mm(bs):
                bs.rb = ps_aux.tile([128, 512], F32, tag="aux", bufs=1,
                                    name="rb")
                nc.tensor.matmul(bs.rb, mask4_sb, bs.r_sb,
                                 start=True, stop=True)

            def t_fold(bs):
                bs.lnr = asb.tile([128, 512], F32, tag="lnr", bufs=2,
                                  name="lnr")
                nc.scalar.activation(bs.lnr, bs.rb, AF.Ln)

            def t_norm(bs):
                bs.rinv = asb.tile([128, 512], F32, tag="rinv", bufs=2,
                                   name="rinv")
                nc.scalar.activation(bs.rinv, bs.lnr, AF.Exp, scale=-1.0)
                bs.att_n = [asb.tile([128, 512], BF16, tag="attsb", bufs=4,
                                     name=f"attn{c2}") for c2 in range(2)]
                for c2 in range(2):
                    nc.vector.tensor_mul(bs.att_n[c2], bs.att_raw[c2],
                                         bs.rinv)

            # Wcx @ x1i is branch-independent: br0 caches it in bf16,
            # br1 skips those matmuls and folds it in on the DVE.
            cpx_sb = [[asb.tile([128, 512], BF16, tag=f"cpx{i}{c}",
                                     name=f"cpx{i}{c}") for c in range(2)]
                      for i in range(NIB)]

            def t_comb(bs, c2):
                cp = ps_aux.tile([128, 512], F32, tag="aux", bufs=1,
                                 name=f"cp{c2}")
                if bs.br == 0:
                    for kc in range(2):
                        nc.tensor.matmul(
                            cp, wcx_sb[:, kc, bass.ts(c2, 128)],
                            x1i_kc[kc][:, bs.isl],
                            start=(kc == 0), stop=(kc == 1))
                    nc.vector.tensor_copy(cpx_sb[bs.ib][c2], cp)
                for kc in range(2):
                    nc.tensor.matmul(
                        cp, wca_sb[:, kc, bass.ts(c2, 128)],
                        bs.att_n[kc],
                        start=(bs.br == 1 and kc == 0), stop=(kc == 1),
                        skip_group_check=True)
                if not hasattr(bs, 'cbb'):
                    bs.cbb = [None, None]
                    bs.absb = [None, None]
                bs.cbb[c2] = asb.tile([128, 512], BF16, tag="cbb", bufs=4,
                                      name=f"cbb{c2}")
                bs.absb[c2] = asb.tile([128, 512], BF16, tag="absb", bufs=4,
                                       name=f"absb{c2}")
                if bs.br == 0:
                    nc.vector.tensor_scalar(bs.cbb[c2], cp,
                                            bce_sb[:, c2:c2 + 1], None,
                                            ALU.add)
                else:
                    nc.vector.scalar_tensor_tensor(
                        bs.cbb[c2], cp, bce_sb[:, c2:c2 + 1],
                        cpx_sb[bs.ib][c2], ALU.add, ALU.add)
                nc.vector.tensor_scalar(
                    bs.absb[c2].bitcast(mybir.dt.uint16),
                    bs.cbb[c2].bitcast(mybir.dt.uint16),
                    0x7FFF, None, ALU.bitwise_and)

            def t_out(bs):
                outp = ps_aux.tile([128, 512], F32, tag="aux", bufs=1,
                                   name="outp")
                for c2 in range(2):
                    nc.tensor.matmul(outp[0:1, :], ones_bf, bs.absb[c2],
                                     start=(c2 == 0), stop=(c2 == 1))
                osb = asb.tile([1, 512], F32, tag="osb", bufs=2, name="osb")
                nc.vector.tensor_copy(osb, outp[0:1, :])
                nc.sync.dma_start(out=out_d[bs.br:bs.br + 1, bs.isl],
                                  in_=osb)

            hooks = {3: t_fold, 4: t_norm,
                     5: lambda bs: t_comb(bs, 0), 6: lambda bs: t_comb(bs, 1),
                     7: t_out}

            # Remaining projections run just-in-time inside the slot
            # stream, staged through the aux PSUM bank, each scheduled a
            # couple of slots before its first consumer.
            half2 = [[x2f_sb[:, kc, jh * IH:(jh + 1) * IH]
                      for kc in range(2)] for jh in range(2)]
            J = lambda h, wt, dst, jb, bias=None: (h, wt, dst, jb, bias)
            proj_sched = {
                (0, 0): [J(half1[0], wk_sb, k4_sb[0][0], 1)],
                (0, 1): [J(half1[0], wk_sb, k4_sb[0][0], 2)],
                (0, 2): [J(half1[0], wk_sb, k4_sb[0][0], 3),
                         J(half1[1], wk_sb, k4_sb[0][1], 0)],
                (0, 3): [J(half1[1], wk_sb, k4_sb[0][1], 1)],
                (0, 4): [J(half1[1], wk_sb, k4_sb[0][1], 2)],
                (0, 5): [J(half1[1], wk_sb, k4_sb[0][1], 3)],
                (0, 6): [J(half1[0], wq_sb, q4_sb[0], 1, bq_sb)],
                (0, 7): [J(half1[0], wq_sb, q4_sb[0], 2, bq_sb)],
                (1, 0): [J(half1[0], wq_sb, q4_sb[0], 3, bq_sb)],
                (1, 1): [J(half2[0], wq_sb, q4_sb[1], 0, bq_sb)],
                (1, 2): [J(half2[0], wk_sb, k4_sb[1][0], 0)],
                (1, 4): [J(half2[0], wk_sb, k4_sb[1][0], 1)],
                (1, 5): [J(half2[0], wk_sb, k4_sb[1][0], 2)],
                (1, 6): [J(half2[0], wk_sb, k4_sb[1][0], 3)],
                (2, 0): [J(half2[1], wk_sb, k4_sb[1][1], 0)],
                (2, 1): [J(half2[1], wk_sb, k4_sb[1][1], 1)],
                (2, 2): [J(half2[1], wk_sb, k4_sb[1][1], 2)],
                (2, 4): [J(half2[1], wk_sb, k4_sb[1][1], 3)],
                (2, 5): [J(half2[0], wq_sb, q4_sb[1], 1, bq_sb)],
                (2, 6): [J(half2[0], wq_sb, q4_sb[1], 2, bq_sb)],
                (3, 0): [J(half2[0], wq_sb, q4_sb[1], 3, bq_sb)],
            }

            def emit_proj(jobs):
                for xf_kc, wt, dst, jb, bias in jobs:
                    sl = bass.ts(jb, 512)
                    kp = ps_aux.tile([128, 512], F32, tag="aux", bufs=1,
                                     name="kp2")
                    for kc in range(2):
                        nc.tensor.matmul(kp, wt[:, kc, :], xf_kc[kc][:, sl],
                                         start=(kc == 0), stop=(kc == 1))
                    if bias is not None:
                        nc.scalar.activation(dst[:, sl], kp, AF.Identity,
                                             bias=bias)
                    else:
                        nc.vector.tensor_copy(dst[:, sl], kp)

            prev = None          # (bs, dg, est) awaiting attended
            done = None          # block whose tail hooks are running
            for bidx, (br, ib) in enumerate(blocks):
                bs = start_block(br, ib)
                for dg in range(NDG):
                    est = scores(bs, dg)
                    if done is not None and dg == 3:
                        t_fold_mm(done)    # before attended: Ln fills the
                                           # ACT gap instead of extending it
                    if prev is not None:
                        pbs, pdg, pest = prev
                        attended(pbs, pdg, pest)
                        if pdg == NDG - 1:
                            end_block(pbs)
                            done = pbs
                    if dg == 0:
                        # after end_block(prev): the DVE FIFO must run
                        # r_copy(prev) before this memset reuses the bank
                        nc.vector.memset(bs.rp, 0.0)
                    prev = (bs, dg, est)
                    if done is not None and dg in hooks:
                        hooks[dg](done)
                    if (bidx, dg) in proj_sched:
                        emit_proj(proj_sched[(bidx, dg)])
            # epilogue: last slot's attended + last block's tail
            pbs, pdg, pest = prev
            attended(pbs, pdg, pest)
            end_block(pbs, last=True)
            t_fold_mm(pbs)
            for dg in (3, 4, 5, 6, 7):
                hooks[dg](pbs)


class _BaccOneActTable(bacc.Bacc):
    """Pin every activation to the natural_log_exp_and_others table set
    (contains Exp, Ln, Abs, Copy and Identity — everything this kernel
    uses).  The default chooser assigns Exp to exp_and_others and Ln to
    natural_log_exp_and_others, reloading tables twice per block (~2.7us
    each on the Scalar engine).  Set indices are preserved so walrus's
    act_func_set_id remap stays valid."""

    def insert_act_table_loads(self):
        import bass_rust as _br
        from concourse.hw_specs import get_activation_tables
        has_activation = any(
            isinstance(i, mybir.InstActivation)
            for b in self.main_func.blocks
            for i in b.instructions
        )
        if not has_activation:
            return
        keep = "natural_log_exp_and_others"
        tables = [(name, funcs if name == keep else set())
                  for name, funcs in
                  get_activation_tables(self.m.arch).items()]
        _br.insert_act_table_loads(self, tables)


_NC_CACHE = {}


def _get_nc():
    if "nc" not in _NC_CACHE:
        nc = _BaccOneActTable(
            "TRN2", debug=False, enable_asserts=False,
            target_bir_lowering=False, enable_partition_id=False)
        with tile.TileContext(nc) as tc:
            build_program(nc, tc)
        nc.compile()
        _NC_CACHE["nc"] = nc
    return _NC_CACHE["nc"]


def host_inputs(x1, x2, Wq, bq, Wk, bk, Wv, bv, Wc, bc):
    """Build the 8 per-core input maps (host-side sharding/layout only)."""
    f = np.float32
    bf = ml_dtypes.bfloat16
    x1 = np.asarray(x1, f); x2 = np.asarray(x2, f)
    Wq = np.asarray(Wq, f); bq = np.asarray(bq, f)
    Wk = np.asarray(Wk, f)
    Wv = np.asarray(Wv, f); bv = np.asarray(bv, f)
    Wc = np.asarray(Wc, f); bc = np.asarray(bc, f)

    # 4x row-replicated q/k projection weights
    Wq4 = np.tile(Wq, (4, 1))            # [128, 256]
    Wk4 = np.tile(Wk, (4, 1))
    wqt = np.ascontiguousarray(Wq4.T.reshape(2, 128, 128)).astype(bf)
    wkt = np.ascontiguousarray(Wk4.T.reshape(2, 128, 128)).astype(bf)
    bq4 = np.tile(bq, 4).reshape(128, 1).copy()
    WcT = np.ascontiguousarray(Wc.T)     # [512, 256]
    wctx = WcT[:C].reshape(2, 128, C).astype(bf)
    # attended weights fold Wv: att-part of combine = (Wc_att @ Wv) @ xen
    wcae = np.ascontiguousarray((Wc[:, C:] @ Wv).T.reshape(2, 128, C)
                                ).astype(bf)
    bce = (bc + Wc[:, C:] @ bv).reshape(2, 128).T.copy()   # [128, 2]
    mask4 = np.zeros((128, 128), ml_dtypes.bfloat16)
    mask4[0::32, :] = 1.0        # fold rows 0/32/64/96 -> all partitions

    def xt_layout(xf):
        # [2,128,N] channel-major -> [2(jh), 128(j in chunk), 16*C] with
        # j on partitions: xt[jh, jl, c16*C:...] = x[:, jh*IH + c16*128+jl]
        xT = xf.reshape(C, N).T                      # [4096, 256]
        xt = xT.reshape(2, 16, 128, C).transpose(0, 2, 1, 3)
        return np.ascontiguousarray(xt.reshape(2, 128, 16 * C)).astype(bf)

    in_maps = []
    for core in range(NCORES):
        b, h = divmod(core, 2)
        x1f = x1[b].reshape(C, N).reshape(2, 128, N)
        x2f = x2[b].reshape(C, N).reshape(2, 128, N)
        if h == 1:   # rotate so this core's query half is columns 0..IH-1
            x1f = np.concatenate([x1f[:, :, IH:], x1f[:, :, :IH]], axis=2)
            x2f = np.concatenate([x2f[:, :, IH:], x2f[:, :, :IH]], axis=2)
        in_maps.append({
            "x1f": np.ascontiguousarray(x1f).astype(bf),
            "x2f": np.ascontiguousarray(x2f).astype(bf),
            "x1t": xt_layout(x1f), "x2t": xt_layout(x2f),
            "wqt": wqt, "wkt": wkt, "wctx": wctx, "wcae": wcae,
            "bq": bq4, "bce": bce, "mask4": mask4,
        })
    return in_maps


def assemble(results):
    """results: list of 8 dicts with 'out' [2, IH] -> (out1, out2) full."""
    outs = []
    for row in range(2):
        full = np.empty((B, 1, HH, WW), np.float32)
        for b in range(B):
            half0 = results[2 * b]["out"][row]
            half1 = results[2 * b + 1]["out"][row]
            full[b, 0] = np.concatenate([half0, half1]).reshape(HH, WW)
        outs.append(full)
    return outs[0], outs[1]


def kernel(x1, x2, Wq, bq, Wk, bk, Wv, bv, Wc, bc):
    in_maps = host_inputs(x1, x2, Wq, bq, Wk, bk, Wv, bv, Wc, bc)
    nc = _get_nc()
    res = run_bass_kernel_spmd(nc, in_maps, core_ids=list(range(NCORES)))
    return assemble(res.results)


# revision 31
# speedup vs baseline: 1.1956x; 1.1956x over previous
"""Trainium2 Bass kernel for nn_CrossAttention (B=4, C=256, H=W=64).

Sharding: 8 cores = (batch b, query-half h). Host permutes each core's
channel-flattened inputs so the core's own query half occupies columns
0..IH-1 (softmax/attended sums are j-order invariant), letting the
query slice and combine input be SBUF views of the full feature load.

Algebraic restructure: att = sum_j v[:,j] E[j,i] with v = Wv x + bv
factors as Wv (X E) + bv r.  After softmax normalization the bv term is
constant (already folded into bce on host), and Wv commutes out of the
attention sum, so the combine weight becomes Wca_eff = Wc[:,C:] @ Wv
(folded on host) and the attended matmuls contract the raw features:
xe[cin,i] = sum_j x[cin,j] E[j,i].  No Wv projection pass at all; the
host ships x transposed (xt tiles, j on partitions) for the attended
stationary operand.

Per core, per branch:
  q = Wq x_i + bq        [32, 2048] 4x row-replicated bf16
  k = Wk x_f             [32, 4096] 4x row-replicated bf16 (bk dropped:
                         constant-in-j shift is softmax-invariant)
  S^T[j, i] = k_j . q_i  FOUR K=32 score matmuls run concurrently in PE
                         row strips (tile_position 0/32/64/96) into one
                         4-bank PSUM tile
  E = exp(S^T) bf16      one ACT op per 4-chunk slot ([128, 2048])
  r[i] = sum_j E[j, i]   ones-matmuls col-packed 4-per-slot at M=1;
                         fold+broadcast via one mask matmul;
                         1/r = exp(-ln r) on ACT (everything runs out
                         of the pinned natural_log_exp table set)
  xe[c, i] = sum_j xt[j, c] E[j, i];  xen = xe / r
  comb = Wcx x_i + Wca_eff xen + bce ; out[i] = sum_c |comb|
                         (bias-add + sign-mask abs on DVE; ones-matmul)
Pipelining: one flat 64-slot stream (8 blocks x 8 groups); slot s runs
scores/exp of s and attended/r of s-1, so block boundaries cost the
same as interior slots.  Each block's softmax-normalize + combine chain
is deferred into the next block's slots (fold at +1, normalize at +2,
combine halves at +4/+5, output reduce at +6) and runs on ACT/DVE in
the shadow of PE work.  attp banks are freed early by raw bf16 copies;
the 1/r chain lives on ACT so the DVE FIFO never blocks next-block
attended matmuls.  PSUM: 4 score + 2 attended + 1 r + 1 rotating aux.
"""

import numpy as np
import ml_dtypes

import concourse.bass as bass
import concourse.bacc as bacc
import concourse.tile as tile
import concourse.mybir as mybir
from concourse.bass_utils import run_bass_kernel_spmd

B, C, HH, WW = 4, 256, 64, 64
N = HH * WW          # 4096
CQK = 32
IH = N // 2          # 2048 query rows per core
NCORES = 8
NJC = N // 128       # 32 key-dim 128-chunks
NDG = NJC // 4       # 8 slots of 4 key-chunks per block
NIB = IH // 512      # 4 query blocks per branch

F32 = mybir.dt.float32
F32R = mybir.dt.float32r
BF16 = mybir.dt.bfloat16
AF = mybir.ActivationFunctionType
ALU = mybir.AluOpType


def build_program(nc, tc):
    # ---- DRAM I/O ------------------------------------------------------
    dram = {}
    for name, shape, dt in [
        ("x1f", [2, 128, N], BF16), ("x2f", [2, 128, N], BF16),
        ("x1t", [2, 128, 16 * C], BF16), ("x2t", [2, 128, 16 * C], BF16),
        ("wqt", [2, 128, 128], BF16), ("wkt", [2, 128, 128], BF16),
        ("wctx", [2, 128, C], BF16), ("wcae", [2, 128, C], BF16),
        ("bq", [128, 1], F32), ("bce", [128, 2], F32),
        ("mask4", [128, 128], BF16),
    ]:
        dram[name] = nc.dram_tensor(name, shape, dt, kind="ExternalInput").ap()
    out_d = nc.dram_tensor("out", [2, IH], F32, kind="ExternalOutput").ap()

    import contextlib
    with contextlib.ExitStack() as ctx:
        persist = ctx.enter_context(tc.tile_pool(name="persist", bufs=1))

        wq_sb = persist.tile([128, 2, 128], BF16, tag="wq")
        wk_sb = persist.tile([128, 2, 128], BF16, tag="wk")
        wcx_sb = persist.tile([128, 2, C], BF16, tag="wcx")
        wca_sb = persist.tile([128, 2, C], BF16, tag="wca")
        bq_sb = persist.tile([128, 1], F32, tag="bq")
        bce_sb = persist.tile([128, 2], F32, tag="bce")
        ones_bf = persist.tile([128, 1], BF16, tag="ones")
        mask4_sb = persist.tile([128, 128], BF16, tag="mask4")

        x1f_sb = persist.tile([128, 2, N], BF16, tag="x1f")
        x2f_sb = persist.tile([128, 2, N], BF16, tag="x2f")
        xt_sb = [[persist.tile([128, 16, C], BF16, tag=f"xt{i}{h}",
                               name=f"xt{i}{h}") for h in range(2)]
                 for i in range(2)]
        q4_sb = [persist.tile([128, IH], BF16, tag=f"q{i}", name=f"q{i}")
                 for i in range(2)]
        k4_sb = [[persist.tile([128, N // 2], BF16, tag=f"k{i}{h}",
                               name=f"k{i}{h}") for h in range(2)]
                 for i in range(2)]

        # DMA order = need order: weights, x1f (q1/k1), x2t (branch-0
        # attended), x2f (q2/k2), x1t (branch-1 attended, needed late)
        for w, t in [("wqt", wq_sb), ("wkt", wk_sb)]:
            for kc in range(2):
                nc.sync.dma_start(out=t[:, kc, :], in_=dram[w][kc])
        nc.sync.dma_start(out=bq_sb, in_=dram["bq"])
        nc.vector.memset(ones_bf, 1.0)
        # xf transfers sliced at projection granularity so the first
        # matmuls start ~15us earlier; xt rides the gpsimd queue
        # ONE queue, strict priority order: queues do not fair-share the
        # DMA engines, so everything rides sync in first-need order.
        def xf_half(dst, src_name, jh):
            for jb in range(4):
                lo = jh * IH + jb * 512
                for kc in range(2):
                    nc.sync.dma_start(
                        out=dst[:, kc, lo:lo + 512],
                        in_=dram[src_name][kc][:, lo:lo + 512])

        xf_half(x1f_sb, "x1f", 0)
        xf_half(x1f_sb, "x1f", 1)
        for w, t in [("wctx", wcx_sb), ("wcae", wca_sb)]:
            for kc in range(2):
                nc.sync.dma_start(out=t[:, kc, :], in_=dram[w][kc])
        nc.sync.dma_start(out=bce_sb, in_=dram["bce"])
        nc.sync.dma_start(out=mask4_sb, in_=dram["mask4"])
        nc.sync.dma_start(out=xt_sb[1][0].rearrange("p a c -> p (a c)"),
                          in_=dram["x2t"][0])
        xf_half(x2f_sb, "x2f", 0)
        nc.sync.dma_start(out=xt_sb[1][1].rearrange("p a c -> p (a c)"),
                          in_=dram["x2t"][1])
        xf_half(x2f_sb, "x2f", 1)
        for jh in range(2):
            nc.sync.dma_start(out=xt_sb[0][jh].rearrange("p a c -> p (a c)"),
                              in_=dram["x1t"][jh])

        x1i_kc = [x1f_sb[:, kc, 0:IH] for kc in range(2)]

        # ---- phase 1: q/k projections (x1 then x2) --------------------
        with tc.tile_pool(name="ps_kq", bufs=1, space="PSUM") as ps_kq:

            def kq_proj(xf_kc, wt, dst, bias, only_jb=None):
                for jb in range(4):
                    if only_jb is not None and jb != only_jb:
                        continue
                    sl = bass.ts(jb, 512)
                    kp = ps_kq.tile([128, 512], F32, tag="kq", bufs=3,
                                    name="kp")
                    for kc in range(2):
                        nc.tensor.matmul(kp, wt[:, kc, :], xf_kc[kc][:, sl],
                                         start=(kc == 0), stop=(kc == 1))
                    if bias is not None:
                        nc.scalar.activation(dst[:, sl], kp, AF.Identity,
                                             bias=bias)
                    elif jb % 2 == 0:
                        nc.scalar.activation(dst[:, sl], kp, AF.Copy)
                    else:
                        nc.vector.tensor_copy(dst[:, sl], kp)

            half1 = [[x1f_sb[:, kc, jh * IH:(jh + 1) * IH]
                      for kc in range(2)] for jh in range(2)]
            kq_proj(half1[0], wq_sb, q4_sb[0], bq_sb, only_jb=0)
            kq_proj(half1[0], wk_sb, k4_sb[0][0], None, only_jb=0)

        with tc.tile_pool(name="attn_sb", bufs=1) as asb, \
             tc.tile_pool(name="ps_st", bufs=1, space="PSUM") as ps_st, \
             tc.tile_pool(name="ps_att", bufs=1, space="PSUM") as ps_att, \
             tc.tile_pool(name="ps_r", bufs=1, space="PSUM") as ps_r, \
             tc.tile_pool(name="ps_aux", bufs=1, space="PSUM") as ps_aux:

            # ---- phase 2: one flat 64-slot pipeline -------------------
            blocks = [(br, ib) for br in range(2) for ib in range(NIB)]

            class Blk:
                pass

            def scores(bs, dg):
                stp = ps_st.tile([128, 4, 512], F32, tag="stp", bufs=1,
                                 name="stp")
                for u in range(4):
                    jc = dg * 4 + u
                    jh, jloc = jc // 16, jc % 16
                    nc.tensor.matmul(
                        stp[:, u, :],
                        bs.k4[jh][32 * u:32 * (u + 1), bass.ts(jloc, 128)],
                        bs.q4[32 * u:32 * (u + 1), bs.isl],
                        start=True, stop=True, tile_position=(32 * u, 0))
                est = asb.tile([128, 4, 512], BF16, tag="est", bufs=3,
                               name="est")
                nc.scalar.activation(
                    est.rearrange("p a n -> p (a n)"),
                    stp.rearrange("p a n -> p (a n)"), AF.Exp)
                return est

            def attended(bs, dg, est):
                for u in range(4):
                    jc = dg * 4 + u
                    jh, jloc = jc // 16, jc % 16
                    for c2 in range(2):
                        nc.tensor.matmul(
                            bs.attp[c2],
                            bs.xt[jh][:, jloc, bass.ds(c2 * 128, 128)],
                            est[:, u, :],
                            start=(dg == 0 and u == 0),
                            stop=(dg == NDG - 1 and u == 3))
                for u in range(4):
                    nc.tensor.matmul(
                        bs.rp[32 * u:32 * u + 1, :], ones_bf, est[:, u, :],
                        start=(dg == 0), stop=(dg == NDG - 1),
                        tile_position=(0, 32 * u))

            def start_block(br, ib):
                bs = Blk()
                bs.br, bs.ib = br, ib
                bs.isl = bass.ts(ib, 512)
                bs.q4, bs.k4 = q4_sb[br], k4_sb[br]
                bs.xt = xt_sb[1 - br]
                bs.attp = [ps_att.tile([128, 512], F32, tag="attp", bufs=2,
                                       name=f"attp{c2}") for c2 in range(2)]
                bs.rp = ps_r.tile([128, 512], F32, tag="rp", bufs=1,
                                  name="rp")
                return bs

            def end_block(bs, last=False):
                """Issued right after attended(dg7): free rp/attp via DVE."""
                bs.r_sb = asb.tile([128, 512], BF16, tag="rsb", bufs=2,
                                   name="r_sb")
                nc.vector.tensor_copy(bs.r_sb, bs.rp)
                if last:      # nothing follows: normalize attp in place
                    bs.att_raw = bs.attp
                    return
                bs.att_raw = [asb.tile([128, 512], BF16, tag="attraw",
                                       bufs=4, name=f"attraw{c2}")
                              for c2 in range(2)]
                for c2 in range(2):
                    nc.vector.tensor_copy(bs.att_raw[c2], bs.attp[c2])

            def t_fold_mm(bs):
                bs.rb = ps_aux.tile([128, 512], F32, tag="aux", bufs=1,
                                    name="rb")
                nc.tensor.matmul(bs.rb, mask4_sb, bs.r_sb,
                                 start=True, stop=True)

            def t_fold(bs):
                bs.lnr = asb.tile([128, 512], F32, tag="lnr", bufs=2,
                                  name="lnr")
                nc.scalar.activation(bs.lnr, bs.rb, AF.Ln)

            def t_norm(bs):
                bs.rinv = asb.tile([128, 512], F32, tag="rinv", bufs=2,
                                   name="rinv")
                nc.scalar.activation(bs.rinv, bs.lnr, AF.Exp, scale=-1.0)
                bs.att_n = [asb.tile([128, 512], BF16, tag="attsb", bufs=4,
                                     name=f"attn{c2}") for c2 in range(2)]
                for c2 in range(2):
                    nc.vector.tensor_mul(bs.att_n[c2], bs.att_raw[c2],
                                         bs.rinv)

            # Wcx @ x1i is branch-independent: br0 caches it in bf16,
            # br1 skips those matmuls and folds it in on the DVE.
            cpx_sb = [[asb.tile([128, 512], BF16, tag=f"cpx{i}{c}",
                                     name=f"cpx{i}{c}") for c in range(2)]
                      for i in range(NIB)]

            def t_comb(bs, c2):
                cp = ps_aux.tile([128, 512], F32, tag="aux", bufs=1,
                                 name=f"cp{c2}")
                if bs.br == 0:
                    for kc in range(2):
                        nc.tensor.matmul(
                            cp, wcx_sb[:, kc, bass.ts(c2, 128)],
                            x1i_kc[kc][:, bs.isl],
                            start=(kc == 0), stop=(kc == 1))
                    nc.vector.tensor_copy(cpx_sb[bs.ib][c2], cp)
                for kc in range(2):
                    nc.tensor.matmul(
                        cp, wca_sb[:, kc, bass.ts(c2, 128)],
                        bs.att_n[kc],
                        start=(bs.br == 1 and kc == 0), stop=(kc == 1),
                        skip_group_check=True)
                if not hasattr(bs, 'cbb'):
                    bs.cbb = [None, None]
                    bs.absb = [None, None]
                bs.cbb[c2] = asb.tile([128, 512], BF16, tag="cbb", bufs=4,
                                      name=f"cbb{c2}")
                bs.absb[c2] = asb.tile([128, 512], BF16, tag="absb", bufs=4,
                                       name=f"absb{c2}")
                if bs.br == 0:
                    nc.vector.tensor_scalar(bs.cbb[c2], cp,
                                            bce_sb[:, c2:c2 + 1], None,
                                            ALU.add)
                else:
                    nc.vector.scalar_tensor_tensor(
                        bs.cbb[c2], cp, bce_sb[:, c2:c2 + 1],
                        cpx_sb[bs.ib][c2], ALU.add, ALU.add)
                nc.vector.tensor_scalar(
                    bs.absb[c2].bitcast(mybir.dt.uint16),
                    bs.cbb[c2].bitcast(mybir.dt.uint16),
                    0x7FFF, None, ALU.bitwise_and)

            def t_out(bs):
                outp = ps_aux.tile([128, 512], F32, tag="aux", bufs=1,
                                   name="outp")
                for c2 in range(2):
                    nc.tensor.matmul(outp[0:1, :], ones_bf, bs.absb[c2],
                                     start=(c2 == 0), stop=(c2 == 1))
                osb = asb.tile([1, 512], F32, tag="osb", bufs=2, name="osb")
                nc.vector.tensor_copy(osb, outp[0:1, :])
                nc.sync.dma_start(out=out_d[bs.br:bs.br + 1, bs.isl],
                                  in_=osb)

            hooks = {3: t_fold, 4: t_norm,
                     5: lambda bs: t_comb(bs, 0), 6: lambda bs: t_comb(bs, 1),
                     7: t_out}

            # Remaining projections run just-in-time inside the slot
            # stream, staged through the aux PSUM bank, each scheduled a
            # couple of slots before its first consumer.
            half2 = [[x2f_sb[:, kc, jh * IH:(jh + 1) * IH]
                      for kc in range(2)] for jh in range(2)]
            J = lambda h, wt, dst, jb, bias=None: (h, wt, dst, jb, bias)
            proj_sched = {
                (0, 0): [J(half1[0], wk_sb, k4_sb[0][0], 1)],
                (0, 1): [J(half1[0], wk_sb, k4_sb[0][0], 2)],
                (0, 2): [J(half1[0], wk_sb, k4_sb[0][0], 3),
                         J(half1[1], wk_sb, k4_sb[0][1], 0)],
                (0, 3): [J(half1[1], wk_sb, k4_sb[0][1], 1)],
                (0, 4): [J(half1[1], wk_sb, k4_sb[0][1], 2)],
                (0, 5): [J(half1[1], wk_sb, k4_sb[0][1], 3)],
                (0, 6): [J(half1[0], wq_sb, q4_sb[0], 1, bq_sb)],
                (0, 7): [J(half1[0], wq_sb, q4_sb[0], 2, bq_sb)],
                (1, 0): [J(half1[0], wq_sb, q4_sb[0], 3, bq_sb)],
                (1, 1): [J(half2[0], wq_sb, q4_sb[1], 0, bq_sb)],
                (1, 2): [J(half2[0], wk_sb, k4_sb[1][0], 0)],
                (1, 4): [J(half2[0], wk_sb, k4_sb[1][0], 1)],
                (1, 5): [J(half2[0], wk_sb, k4_sb[1][0], 2)],
                (1, 6): [J(half2[0], wk_sb, k4_sb[1][0], 3)],
                (2, 0): [J(half2[1], wk_sb, k4_sb[1][1], 0)],
                (2, 1): [J(half2[1], wk_sb, k4_sb[1][1], 1)],
                (2, 2): [J(half2[1], wk_sb, k4_sb[1][1], 2)],
                (2, 4): [J(half2[1], wk_sb, k4_sb[1][1], 3)],
                (2, 5): [J(half2[0], wq_sb, q4_sb[1], 1, bq_sb)],
                (2, 6): [J(half2[0], wq_sb, q4_sb[1], 2, bq_sb)],
                (3, 0): [J(half2[0], wq_sb, q4_sb[1], 3, bq_sb)],
            }

            def emit_proj(jobs):
                for xf_kc, wt, dst, jb, bias in jobs:
                    sl = bass.ts(jb, 512)
                    kp = ps_aux.tile([128, 512], F32, tag="aux", bufs=1,
                                     name="kp2")
                    for kc in range(2):
                        nc.tensor.matmul(kp, wt[:, kc, :], xf_kc[kc][:, sl],
                                         start=(kc == 0), stop=(kc == 1))
                    if bias is not None:
                        nc.scalar.activation(dst[:, sl], kp, AF.Identity,
                                             bias=bias)
                    else:
                        nc.vector.tensor_copy(dst[:, sl], kp)

            prev = None          # (bs, dg, est) awaiting attended
            done = None          # block whose tail hooks are running
            for bidx, (br, ib) in enumerate(blocks):
                bs = start_block(br, ib)
                for dg in range(NDG):
                    est = scores(bs, dg)
                    if done is not None and dg == 3:
                        t_fold_mm(done)    # before attended: Ln fills the
                                           # ACT gap instead of extending it
                    if prev is not None:
                        pbs, pdg, pest = prev
                        attended(pbs, pdg, pest)
                        if pdg == NDG - 1:
                            end_block(pbs)
                            done = pbs
                    if dg == 0:
                        # after end_block(prev): the DVE FIFO must run
                        # r_copy(prev) before this memset reuses the bank
                        nc.vector.memset(bs.rp, 0.0)
                    prev = (bs, dg, est)
                    if done is not None and dg in hooks:
                        hooks[dg](done)
                    if (bidx, dg) in proj_sched:
                        emit_proj(proj_sched[(bidx, dg)])
            # epilogue: last slot's attended + last block's tail
            pbs, pdg, pest = prev
            attended(pbs, pdg, pest)
            end_block(pbs, last=True)
            t_fold_mm(pbs)
            for dg in (3, 4, 5, 6, 7):
                hooks[dg](pbs)


class _BaccOneActTable(bacc.Bacc):
    """Pin every activation to the natural_log_exp_and_others table set
    (contains Exp, Ln, Abs, Copy and Identity — everything this kernel
    uses).  The default chooser assigns Exp to exp_and_others and Ln to
    natural_log_exp_and_others, reloading tables twice per block (~2.7us
    each on the Scalar engine).  Set indices are preserved so walrus's
    act_func_set_id remap stays valid."""

    def insert_act_table_loads(self):
        import bass_rust as _br
        from concourse.hw_specs import get_activation_tables
        has_activation = any(
            isinstance(i, mybir.InstActivation)
            for b in self.main_func.blocks
            for i in b.instructions
        )
        if not has_activation:
            return
        keep = "natural_log_exp_and_others"
        tables = [(name, funcs if name == keep else set())
                  for name, funcs in
                  get_activation_tables(self.m.arch).items()]
        _br.insert_act_table_loads(self, tables)


_NC_CACHE = {}


def _get_nc():
    if "nc" not in _NC_CACHE:
        nc = _BaccOneActTable(
            "TRN2", debug=False, enable_asserts=False,
            target_bir_lowering=False, enable_partition_id=False)
        with tile.TileContext(nc) as tc:
            build_program(nc, tc)
        nc.compile()
        _NC_CACHE["nc"] = nc
    return _NC_CACHE["nc"]


def host_inputs(x1, x2, Wq, bq, Wk, bk, Wv, bv, Wc, bc):
    """Build the 8 per-core input maps (host-side sharding/layout only)."""
    f = np.float32
    bf = ml_dtypes.bfloat16
    x1 = np.asarray(x1, f); x2 = np.asarray(x2, f)
    Wq = np.asarray(Wq, f); bq = np.asarray(bq, f)
    Wk = np.asarray(Wk, f)
    Wv = np.asarray(Wv, f); bv = np.asarray(bv, f)
    Wc = np.asarray(Wc, f); bc = np.asarray(bc, f)

    # 4x row-replicated q/k projection weights
    Wq4 = np.tile(Wq, (4, 1))            # [128, 256]
    Wk4 = np.tile(Wk, (4, 1))
    wqt = np.ascontiguousarray(Wq4.T.reshape(2, 128, 128)).astype(bf)
    wkt = np.ascontiguousarray(Wk4.T.reshape(2, 128, 128)).astype(bf)
    bq4 = np.tile(bq, 4).reshape(128, 1).copy()
    WcT = np.ascontiguousarray(Wc.T)     # [512, 256]
    wctx = WcT[:C].reshape(2, 128, C).astype(bf)
    # attended weights fold Wv: att-part of combine = (Wc_att @ Wv) @ xen
    wcae = np.ascontiguousarray((Wc[:, C:] @ Wv).T.reshape(2, 128, C)
                                ).astype(bf)
    bce = (bc + Wc[:, C:] @ bv).reshape(2, 128).T.copy()   # [128, 2]
    mask4 = np.zeros((128, 128), ml_dtypes.bfloat16)
    mask4[0::32, :] = 1.0        # fold rows 0/32/64/96 -> all partitions

    def xt_layout(xf):
        # [2,128,N] channel-major -> [2(jh), 128(j in chunk), 16*C] with
        # j on partitions: xt[jh, jl, c16*C:...] = x[:, jh*IH + c16*128+jl]
        xT = xf.reshape(C, N).T                      # [4096, 256]
        xt = xT.reshape(2, 16, 128, C).transpose(0, 2, 1, 3)
        return np.ascontiguousarray(xt.reshape(2, 128, 16 * C)).astype(bf)

    in_maps = []
    for core in range(NCORES):
        b, h = divmod(core, 2)
        x1f = x1[b].reshape(C, N).reshape(2, 128, N)
        x2f = x2[b].reshape(C, N).reshape(2, 128, N)
        if h == 1:   # rotate so this core's query half is columns 0..IH-1
            x1f = np.concatenate([x1f[:, :, IH:], x1f[:, :, :IH]], axis=2)
            x2f = np.concatenate([x2f[:, :, IH:], x2f[:, :, :IH]], axis=2)
        in_maps.append({
            "x1f": np.ascontiguousarray(x1f).astype(bf),
            "x2f": np.ascontiguousarray(x2f).astype(bf),
            "x1t": xt_layout(x1f), "x2t": xt_layout(x2f),
            "wqt": wqt, "wkt": wkt, "wctx": wctx, "wcae": wcae,
            "bq": bq4, "bce": bce, "mask4": mask4,
        })
    return in_maps


def assemble(results):
    """results: list of 8 dicts with 'out' [2, IH] -> (out1, out2) full."""
    outs = []
    for row in range(2):
        full = np.empty((B, 1, HH, WW), np.float32)
        for b in range(B):
            half0 = results[2 * b]["out"][row]
            half1 = results[2 * b + 1]["out"][row]
            full[b, 0] = np.concatenate([half0, half1]).reshape(HH, WW)
        outs.append(full)
    return outs[0], outs[1]


def kernel(x1, x2, Wq, bq, Wk, bk, Wv, bv, Wc, bc):
    in_maps = host_inputs(x1, x2, Wq, bq, Wk, bk, Wv, bv, Wc, bc)
    nc = _get_nc()
    res = run_bass_kernel_spmd(nc, in_maps, core_ids=list(range(NCORES)))
    return assemble(res.results)


# revision 32
# speedup vs baseline: 1.1972x; 1.0013x over previous
"""Trainium2 Bass kernel for nn_CrossAttention (B=4, C=256, H=W=64).

Sharding: 8 cores = (batch b, query-half h). Host permutes each core's
channel-flattened inputs so the core's own query half occupies columns
0..IH-1 (softmax/attended sums are j-order invariant), letting the
query slice and combine input be SBUF views of the full feature load.

Algebraic restructure: att = sum_j v[:,j] E[j,i] with v = Wv x + bv
factors as Wv (X E) + bv r.  After softmax normalization the bv term is
constant (already folded into bce on host), and Wv commutes out of the
attention sum, so the combine weight becomes Wca_eff = Wc[:,C:] @ Wv
(folded on host) and the attended matmuls contract the raw features:
xe[cin,i] = sum_j x[cin,j] E[j,i].  No Wv projection pass at all; the
host ships x transposed (xt tiles, j on partitions) for the attended
stationary operand.

Per core, per branch:
  q = Wq x_i + bq        [32, 2048] 4x row-replicated bf16
  k = Wk x_f             [32, 4096] 4x row-replicated bf16 (bk dropped:
                         constant-in-j shift is softmax-invariant)
  S^T[j, i] = k_j . q_i  FOUR K=32 score matmuls run concurrently in PE
                         row strips (tile_position 0/32/64/96) into one
                         4-bank PSUM tile
  E = exp(S^T) bf16      one ACT op per 4-chunk slot ([128, 2048])
  r[i] = sum_j E[j, i]   ones-matmuls col-packed 4-per-slot at M=1;
                         fold+broadcast via one mask matmul;
                         1/r = exp(-ln r) on ACT (everything runs out
                         of the pinned natural_log_exp table set)
  xe[c, i] = sum_j xt[j, c] E[j, i];  xen = xe / r
  comb = Wcx x_i + Wca_eff xen + bce ; out[i] = sum_c |comb|
                         (bias-add + sign-mask abs on DVE; ones-matmul)
Pipelining: one flat 64-slot stream (8 blocks x 8 groups); slot s runs
scores/exp of s and attended/r of s-1, so block boundaries cost the
same as interior slots.  Each block's softmax-normalize + combine chain
is deferred into the next block's slots (fold at +1, normalize at +2,
combine halves at +4/+5, output reduce at +6) and runs on ACT/DVE in
the shadow of PE work.  attp banks are freed early by raw bf16 copies;
the 1/r chain lives on ACT so the DVE FIFO never blocks next-block
attended matmuls.  PSUM: 4 score + 2 attended + 1 r + 1 rotating aux.
"""

import numpy as np
import ml_dtypes

import concourse.bass as bass
import concourse.bacc as bacc
import concourse.tile as tile
import concourse.mybir as mybir
from concourse.bass_utils import run_bass_kernel_spmd

B, C, HH, WW = 4, 256, 64, 64
N = HH * WW          # 4096
CQK = 32
IH = N // 2          # 2048 query rows per core
NCORES = 8
NJC = N // 128       # 32 key-dim 128-chunks
NDG = NJC // 4       # 8 slots of 4 key-chunks per block
NIB = IH // 512      # 4 query blocks per branch

F32 = mybir.dt.float32
F32R = mybir.dt.float32r
BF16 = mybir.dt.bfloat16
AF = mybir.ActivationFunctionType
ALU = mybir.AluOpType


def build_program(nc, tc):
    # ---- DRAM I/O ------------------------------------------------------
    dram = {}
    for name, shape, dt in [
        ("x1f", [2, 128, N], BF16), ("x2f", [2, 128, N], BF16),
        ("x1t", [2, 128, 16 * C], BF16), ("x2t", [2, 128, 16 * C], BF16),
        ("wqt", [2, 128, 128], BF16), ("wkt", [2, 128, 128], BF16),
        ("wctx", [2, 128, C], BF16), ("wcae", [2, 128, C], BF16),
        ("bq", [128, 1], F32), ("bce", [128, 2], F32),
        ("mask4", [128, 128], BF16),
    ]:
        dram[name] = nc.dram_tensor(name, shape, dt, kind="ExternalInput").ap()
    out_d = nc.dram_tensor("out", [2, IH], F32, kind="ExternalOutput").ap()

    import contextlib
    with contextlib.ExitStack() as ctx:
        persist = ctx.enter_context(tc.tile_pool(name="persist", bufs=1))

        wq_sb = persist.tile([128, 2, 128], BF16, tag="wq")
        wk_sb = persist.tile([128, 2, 128], BF16, tag="wk")
        wcx_sb = persist.tile([128, 2, C], BF16, tag="wcx")
        wca_sb = persist.tile([128, 2, C], BF16, tag="wca")
        bq_sb = persist.tile([128, 1], F32, tag="bq")
        bce_sb = persist.tile([128, 2], F32, tag="bce")
        ones_bf = persist.tile([128, 1], BF16, tag="ones")
        mask4_sb = persist.tile([128, 128], BF16, tag="mask4")

        x1f_sb = persist.tile([128, 2, N], BF16, tag="x1f")
        x2f_sb = persist.tile([128, 2, N], BF16, tag="x2f")
        xt_sb = [[persist.tile([128, 16, C], BF16, tag=f"xt{i}{h}",
                               name=f"xt{i}{h}") for h in range(2)]
                 for i in range(2)]
        q4_sb = [persist.tile([128, IH], BF16, tag=f"q{i}", name=f"q{i}")
                 for i in range(2)]
        k4_sb = [[persist.tile([128, N // 2], BF16, tag=f"k{i}{h}",
                               name=f"k{i}{h}") for h in range(2)]
                 for i in range(2)]

        # DMA order = need order: weights, x1f (q1/k1), x2t (branch-0
        # attended), x2f (q2/k2), x1t (branch-1 attended, needed late)
        for w, t in [("wqt", wq_sb), ("wkt", wk_sb)]:
            for kc in range(2):
                nc.sync.dma_start(out=t[:, kc, :], in_=dram[w][kc])
        nc.sync.dma_start(out=bq_sb, in_=dram["bq"])
        nc.vector.memset(ones_bf, 1.0)
        # xf transfers sliced at projection granularity so the first
        # matmuls start ~15us earlier; xt rides the gpsimd queue
        # ONE queue, strict priority order: queues do not fair-share the
        # DMA engines, so everything rides sync in first-need order.
        def xf_half(dst, src_name, jh):
            for jb in range(4):
                lo = jh * IH + jb * 512
                for kc in range(2):
                    nc.sync.dma_start(
                        out=dst[:, kc, lo:lo + 512],
                        in_=dram[src_name][kc][:, lo:lo + 512])

        xf_half(x1f_sb, "x1f", 0)
        xf_half(x1f_sb, "x1f", 1)
        for w, t in [("wctx", wcx_sb), ("wcae", wca_sb)]:
            for kc in range(2):
                nc.sync.dma_start(out=t[:, kc, :], in_=dram[w][kc])
        nc.sync.dma_start(out=bce_sb, in_=dram["bce"])
        nc.sync.dma_start(out=mask4_sb, in_=dram["mask4"])
        nc.sync.dma_start(out=xt_sb[1][0].rearrange("p a c -> p (a c)"),
                          in_=dram["x2t"][0])
        xf_half(x2f_sb, "x2f", 0)
        nc.sync.dma_start(out=xt_sb[1][1].rearrange("p a c -> p (a c)"),
                          in_=dram["x2t"][1])
        xf_half(x2f_sb, "x2f", 1)
        for jh in range(2):
            nc.sync.dma_start(out=xt_sb[0][jh].rearrange("p a c -> p (a c)"),
                              in_=dram["x1t"][jh])

        x1i_kc = [x1f_sb[:, kc, 0:IH] for kc in range(2)]

        # ---- phase 1: q/k projections (x1 then x2) --------------------
        with tc.tile_pool(name="ps_kq", bufs=1, space="PSUM") as ps_kq:

            def kq_proj(xf_kc, wt, dst, bias, only_jb=None):
                for jb in range(4):
                    if only_jb is not None and jb != only_jb:
                        continue
                    sl = bass.ts(jb, 512)
                    kp = ps_kq.tile([128, 512], F32, tag="kq", bufs=3,
                                    name="kp")
                    for kc in range(2):
                        nc.tensor.matmul(kp, wt[:, kc, :], xf_kc[kc][:, sl],
                                         start=(kc == 0), stop=(kc == 1))
                    if bias is not None:
                        nc.scalar.activation(dst[:, sl], kp, AF.Identity,
                                             bias=bias)
                    elif jb % 2 == 0:
                        nc.scalar.activation(dst[:, sl], kp, AF.Copy)
                    else:
                        nc.vector.tensor_copy(dst[:, sl], kp)

            half1 = [[x1f_sb[:, kc, jh * IH:(jh + 1) * IH]
                      for kc in range(2)] for jh in range(2)]
            kq_proj(half1[0], wq_sb, q4_sb[0], bq_sb, only_jb=0)
            kq_proj(half1[0], wk_sb, k4_sb[0][0], None, only_jb=0)

        with tc.tile_pool(name="attn_sb", bufs=1) as asb, \
             tc.tile_pool(name="ps_st", bufs=1, space="PSUM") as ps_st, \
             tc.tile_pool(name="ps_att", bufs=1, space="PSUM") as ps_att, \
             tc.tile_pool(name="ps_r", bufs=1, space="PSUM") as ps_r, \
             tc.tile_pool(name="ps_aux", bufs=1, space="PSUM") as ps_aux:

            # ---- phase 2: one flat 64-slot pipeline -------------------
            blocks = [(br, ib) for br in range(2) for ib in range(NIB)]

            class Blk:
                pass

            def scores(bs, dg):
                stp = ps_st.tile([128, 4, 512], F32, tag="stp", bufs=1,
                                 name="stp")
                for u in range(4):
                    jc = dg * 4 + u
                    jh, jloc = jc // 16, jc % 16
                    nc.tensor.matmul(
                        stp[:, u, :],
                        bs.k4[jh][32 * u:32 * (u + 1), bass.ts(jloc, 128)],
                        bs.q4[32 * u:32 * (u + 1), bs.isl],
                        start=True, stop=True, tile_position=(32 * u, 0))
                est = asb.tile([128, 4, 512], BF16, tag="est", bufs=3,
                               name="est")
                nc.scalar.activation(
                    est.rearrange("p a n -> p (a n)"),
                    stp.rearrange("p a n -> p (a n)"), AF.Exp)
                return est

            def attended(bs, dg, est):
                for u in range(4):
                    jc = dg * 4 + u
                    jh, jloc = jc // 16, jc % 16
                    for c2 in range(2):
                        nc.tensor.matmul(
                            bs.attp[c2],
                            bs.xt[jh][:, jloc, bass.ds(c2 * 128, 128)],
                            est[:, u, :],
                            start=(dg == 0 and u == 0),
                            stop=(dg == NDG - 1 and u == 3))
                for u in range(4):
                    nc.tensor.matmul(
                        bs.rp[32 * u:32 * u + 1, :], ones_bf, est[:, u, :],
                        start=(dg == 0), stop=(dg == NDG - 1),
                        tile_position=(0, 32 * u))

            def start_block(br, ib):
                bs = Blk()
                bs.br, bs.ib = br, ib
                bs.isl = bass.ts(ib, 512)
                bs.q4, bs.k4 = q4_sb[br], k4_sb[br]
                bs.xt = xt_sb[1 - br]
                bs.attp = [ps_att.tile([128, 512], F32, tag="attp", bufs=2,
                                       name=f"attp{c2}") for c2 in range(2)]
                bs.rp = ps_r.tile([128, 512], F32, tag="rp", bufs=1,
                                  name="rp")
                return bs

            def end_block(bs, last=False):
                """Issued right after attended(dg7): free rp/attp via DVE."""
                bs.r_sb = asb.tile([128, 512], BF16, tag="rsb", bufs=2,
                                   name="r_sb")
                nc.vector.tensor_copy(bs.r_sb, bs.rp)
                if last:      # nothing follows: normalize attp in place
                    bs.att_raw = bs.attp
                    return
                bs.att_raw = [asb.tile([128, 512], BF16, tag="attraw",
                                       bufs=4, name=f"attraw{c2}")
                              for c2 in range(2)]
                for c2 in range(2):
                    nc.vector.tensor_copy(bs.att_raw[c2], bs.attp[c2])
                # next block's r bank cleared after the attp-freeing casts
                if bs.nxt is not None:
                    nc.vector.memset(bs.nxt.rp, 0.0)

            def t_fold_mm(bs):
                bs.rb = ps_aux.tile([128, 512], F32, tag="aux", bufs=1,
                                    name="rb")
                nc.tensor.matmul(bs.rb, mask4_sb, bs.r_sb,
                                 start=True, stop=True)

            def t_fold(bs):
                bs.lnr = asb.tile([128, 512], F32, tag="lnr", bufs=2,
                                  name="lnr")
                nc.scalar.activation(bs.lnr, bs.rb, AF.Ln)

            def t_norm(bs):
                bs.rinv = asb.tile([128, 512], F32, tag="rinv", bufs=2,
                                   name="rinv")
                nc.scalar.activation(bs.rinv, bs.lnr, AF.Exp, scale=-1.0)
                bs.att_n = [asb.tile([128, 512], BF16, tag="attsb", bufs=4,
                                     name=f"attn{c2}") for c2 in range(2)]
                for c2 in range(2):
                    nc.vector.tensor_mul(bs.att_n[c2], bs.att_raw[c2],
                                         bs.rinv)

            # Wcx @ x1i is branch-independent: br0 caches it in bf16,
            # br1 skips those matmuls and folds it in on the DVE.
            cpx_sb = [[asb.tile([128, 512], BF16, tag=f"cpx{i}{c}",
                                     name=f"cpx{i}{c}") for c in range(2)]
                      for i in range(NIB)]

            def t_comb(bs, c2):
                cp = ps_aux.tile([128, 512], F32, tag="aux", bufs=1,
                                 name=f"cp{c2}")
                if bs.br == 0:
                    for kc in range(2):
                        nc.tensor.matmul(
                            cp, wcx_sb[:, kc, bass.ts(c2, 128)],
                            x1i_kc[kc][:, bs.isl],
                            start=(kc == 0), stop=(kc == 1))
                    nc.vector.tensor_copy(cpx_sb[bs.ib][c2], cp)
                for kc in range(2):
                    nc.tensor.matmul(
                        cp, wca_sb[:, kc, bass.ts(c2, 128)],
                        bs.att_n[kc],
                        start=(bs.br == 1 and kc == 0), stop=(kc == 1),
                        skip_group_check=True)
                if not hasattr(bs, 'cbb'):
                    bs.cbb = [None, None]
                    bs.absb = [None, None]
                bs.cbb[c2] = asb.tile([128, 512], BF16, tag="cbb", bufs=4,
                                      name=f"cbb{c2}")
                bs.absb[c2] = asb.tile([128, 512], BF16, tag="absb", bufs=4,
                                       name=f"absb{c2}")
                if bs.br == 0:
                    nc.vector.tensor_scalar(bs.cbb[c2], cp,
                                            bce_sb[:, c2:c2 + 1], None,
                                            ALU.add)
                else:
                    nc.vector.scalar_tensor_tensor(
                        bs.cbb[c2], cp, bce_sb[:, c2:c2 + 1],
                        cpx_sb[bs.ib][c2], ALU.add, ALU.add)
                nc.vector.tensor_scalar(
                    bs.absb[c2].bitcast(mybir.dt.uint16),
                    bs.cbb[c2].bitcast(mybir.dt.uint16),
                    0x7FFF, None, ALU.bitwise_and)

            def t_out(bs):
                outp = ps_aux.tile([128, 512], F32, tag="aux", bufs=1,
                                   name="outp")
                for c2 in range(2):
                    nc.tensor.matmul(outp[0:1, :], ones_bf, bs.absb[c2],
                                     start=(c2 == 0), stop=(c2 == 1))
                osb = asb.tile([1, 512], F32, tag="osb", bufs=2, name="osb")
                nc.vector.tensor_copy(osb, outp[0:1, :])
                nc.sync.dma_start(out=out_d[bs.br:bs.br + 1, bs.isl],
                                  in_=osb)

            hooks = {3: t_fold, 4: t_norm,
                     5: lambda bs: t_comb(bs, 0), 6: lambda bs: t_comb(bs, 1),
                     7: t_out}

            # Remaining projections run just-in-time inside the slot
            # stream, staged through the aux PSUM bank, each scheduled a
            # couple of slots before its first consumer.
            half2 = [[x2f_sb[:, kc, jh * IH:(jh + 1) * IH]
                      for kc in range(2)] for jh in range(2)]
            J = lambda h, wt, dst, jb, bias=None: (h, wt, dst, jb, bias)
            proj_sched = {
                (0, 0): [J(half1[0], wk_sb, k4_sb[0][0], 1)],
                (0, 1): [J(half1[0], wk_sb, k4_sb[0][0], 2)],
                (0, 2): [J(half1[0], wk_sb, k4_sb[0][0], 3),
                         J(half1[1], wk_sb, k4_sb[0][1], 0)],
                (0, 3): [J(half1[1], wk_sb, k4_sb[0][1], 1)],
                (0, 4): [J(half1[1], wk_sb, k4_sb[0][1], 2)],
                (0, 5): [J(half1[1], wk_sb, k4_sb[0][1], 3)],
                (0, 6): [J(half1[0], wq_sb, q4_sb[0], 1, bq_sb)],
                (0, 7): [J(half1[0], wq_sb, q4_sb[0], 2, bq_sb)],
                (1, 0): [J(half1[0], wq_sb, q4_sb[0], 3, bq_sb)],
                (1, 1): [J(half2[0], wq_sb, q4_sb[1], 0, bq_sb)],
                (1, 2): [J(half2[0], wk_sb, k4_sb[1][0], 0)],
                (1, 4): [J(half2[0], wk_sb, k4_sb[1][0], 1)],
                (1, 5): [J(half2[0], wk_sb, k4_sb[1][0], 2)],
                (1, 6): [J(half2[0], wk_sb, k4_sb[1][0], 3)],
                (2, 0): [J(half2[1], wk_sb, k4_sb[1][1], 0)],
                (2, 1): [J(half2[1], wk_sb, k4_sb[1][1], 1)],
                (2, 2): [J(half2[1], wk_sb, k4_sb[1][1], 2)],
                (2, 4): [J(half2[1], wk_sb, k4_sb[1][1], 3)],
                (2, 5): [J(half2[0], wq_sb, q4_sb[1], 1, bq_sb)],
                (2, 6): [J(half2[0], wq_sb, q4_sb[1], 2, bq_sb)],
                (3, 0): [J(half2[0], wq_sb, q4_sb[1], 3, bq_sb)],
            }

            def emit_proj(jobs):
                for xf_kc, wt, dst, jb, bias in jobs:
                    sl = bass.ts(jb, 512)
                    kp = ps_aux.tile([128, 512], F32, tag="aux", bufs=1,
                                     name="kp2")
                    for kc in range(2):
                        nc.tensor.matmul(kp, wt[:, kc, :], xf_kc[kc][:, sl],
                                         start=(kc == 0), stop=(kc == 1))
                    if bias is not None:
                        nc.scalar.activation(dst[:, sl], kp, AF.Identity,
                                             bias=bias)
                    else:
                        nc.vector.tensor_copy(dst[:, sl], kp)

            prev = None          # (bs, dg, est) awaiting attended
            done = None          # block whose tail hooks are running
            for bidx, (br, ib) in enumerate(blocks):
                bs = start_block(br, ib)
                if bidx == 0:
                    nc.vector.memset(bs.rp, 0.0)
                for dg in range(NDG):
                    est = scores(bs, dg)
                    if done is not None and dg == 3:
                        t_fold_mm(done)    # before attended: Ln fills the
                                           # ACT gap instead of extending it
                    if prev is not None:
                        pbs, pdg, pest = prev
                        attended(pbs, pdg, pest)
                        if pdg == NDG - 1:
                            pbs.nxt = bs
                            end_block(pbs)
                            done = pbs
                    prev = (bs, dg, est)
                    if done is not None and dg in hooks:
                        hooks[dg](done)
                    if (bidx, dg) in proj_sched:
                        emit_proj(proj_sched[(bidx, dg)])
            # epilogue: last slot's attended + last block's tail
            pbs, pdg, pest = prev
            attended(pbs, pdg, pest)
            pbs.nxt = None
            end_block(pbs, last=True)
            t_fold_mm(pbs)
            for dg in (3, 4, 5, 6, 7):
                hooks[dg](pbs)


class _BaccOneActTable(bacc.Bacc):
    """Pin every activation to the natural_log_exp_and_others table set
    (contains Exp, Ln, Abs, Copy and Identity — everything this kernel
    uses).  The default chooser assigns Exp to exp_and_others and Ln to
    natural_log_exp_and_others, reloading tables twice per block (~2.7us
    each on the Scalar engine).  Set indices are preserved so walrus's
    act_func_set_id remap stays valid."""

    def insert_act_table_loads(self):
        import bass_rust as _br
        from concourse.hw_specs import get_activation_tables
        has_activation = any(
            isinstance(i, mybir.InstActivation)
            for b in self.main_func.blocks
            for i in b.instructions
        )
        if not has_activation:
            return
        keep = "natural_log_exp_and_others"
        tables = [(name, funcs if name == keep else set())
                  for name, funcs in
                  get_activation_tables(self.m.arch).items()]
        _br.insert_act_table_loads(self, tables)


_NC_CACHE = {}


def _get_nc():
    if "nc" not in _NC_CACHE:
        nc = _BaccOneActTable(
            "TRN2", debug=False, enable_asserts=False,
            target_bir_lowering=False, enable_partition_id=False)
        with tile.TileContext(nc) as tc:
            build_program(nc, tc)
        nc.compile()
        _NC_CACHE["nc"] = nc
    return _NC_CACHE["nc"]


def host_inputs(x1, x2, Wq, bq, Wk, bk, Wv, bv, Wc, bc):
    """Build the 8 per-core input maps (host-side sharding/layout only)."""
    f = np.float32
    bf = ml_dtypes.bfloat16
    x1 = np.asarray(x1, f); x2 = np.asarray(x2, f)
    Wq = np.asarray(Wq, f); bq = np.asarray(bq, f)
    Wk = np.asarray(Wk, f)
    Wv = np.asarray(Wv, f); bv = np.asarray(bv, f)
    Wc = np.asarray(Wc, f); bc = np.asarray(bc, f)

    # 4x row-replicated q/k projection weights
    Wq4 = np.tile(Wq, (4, 1))            # [128, 256]
    Wk4 = np.tile(Wk, (4, 1))
    wqt = np.ascontiguousarray(Wq4.T.reshape(2, 128, 128)).astype(bf)
    wkt = np.ascontiguousarray(Wk4.T.reshape(2, 128, 128)).astype(bf)
    bq4 = np.tile(bq, 4).reshape(128, 1).copy()
    WcT = np.ascontiguousarray(Wc.T)     # [512, 256]
    wctx = WcT[:C].reshape(2, 128, C).astype(bf)
    # attended weights fold Wv: att-part of combine = (Wc_att @ Wv) @ xen
    wcae = np.ascontiguousarray((Wc[:, C:] @ Wv).T.reshape(2, 128, C)
                                ).astype(bf)
    bce = (bc + Wc[:, C:] @ bv).reshape(2, 128).T.copy()   # [128, 2]
    mask4 = np.zeros((128, 128), ml_dtypes.bfloat16)
    mask4[0::32, :] = 1.0        # fold rows 0/32/64/96 -> all partitions

    def xt_layout(xf):
        # [2,128,N] channel-major -> [2(jh), 128(j in chunk), 16*C] with
        # j on partitions: xt[jh, jl, c16*C:...] = x[:, jh*IH + c16*128+jl]
        xT = xf.reshape(C, N).T                      # [4096, 256]
        xt = xT.reshape(2, 16, 128, C).transpose(0, 2, 1, 3)
        return np.ascontiguousarray(xt.reshape(2, 128, 16 * C)).astype(bf)

    in_maps = []
    for core in range(NCORES):
        b, h = divmod(core, 2)
        x1f = x1[b].reshape(C, N).reshape(2, 128, N)
        x2f = x2[b].reshape(C, N).reshape(2, 128, N)
        if h == 1:   # rotate so this core's query half is columns 0..IH-1
            x1f = np.concatenate([x1f[:, :, IH:], x1f[:, :, :IH]], axis=2)
            x2f = np.concatenate([x2f[:, :, IH:], x2f[:, :, :IH]], axis=2)
        in_maps.append({
            "x1f": np.ascontiguousarray(x1f).astype(bf),
            "x2f": np.ascontiguousarray(x2f).astype(bf),
            "x1t": xt_layout(x1f), "x2t": xt_layout(x2f),
            "wqt": wqt, "wkt": wkt, "wctx": wctx, "wcae": wcae,
            "bq": bq4, "bce": bce, "mask4": mask4,
        })
    return in_maps


def assemble(results):
    """results: list of 8 dicts with 'out' [2, IH] -> (out1, out2) full."""
    outs = []
    for row in range(2):
        full = np.empty((B, 1, HH, WW), np.float32)
        for b in range(B):
            half0 = results[2 * b]["out"][row]
            half1 = results[2 * b + 1]["out"][row]
            full[b, 0] = np.concatenate([half0, half1]).reshape(HH, WW)
        outs.append(full)
    return outs[0], outs[1]


def kernel(x1, x2, Wq, bq, Wk, bk, Wv, bv, Wc, bc):
    in_maps = host_inputs(x1, x2, Wq, bq, Wk, bk, Wv, bv, Wc, bc)
    nc = _get_nc()
    res = run_bass_kernel_spmd(nc, in_maps, core_ids=list(range(NCORES)))
    return assemble(res.results)
